# revision 38
# baseline (speedup 1.0000x reference)
"""Trainium2 Bass kernel for nn_MixedAttnHeadEmbed_82076825027210.

Computes, per batch element:
    out = sum over h in {4, 8, 12} of CausalAttention(Q_mix_h, K_mix_h, V_mix_h)
where Q/K/V_mix_h are weighted mixtures (9 scalar weights) of head-sliced
views of x's q/k/v channel groups, padded per head to hd = 768/h.

Sharding: data-parallel over batch B=8 across the 8 NeuronCores (one batch
element per core); the 9 mixture weights are baked into the compiled program
as immediates.

Per-core engine assignment (engine-busy budget vs the ~119us ACT floor):
  ACT  exp only -- the hard floor: 92us of element time + per-instr init
  PE   S^T chunks, diagonal-mask matmuls, PV (+l via ones column)
  DVE  Q/K/V mixing (tensor_scalar@4x + tensor_tensor adds@2x)
  Pool normalize: rec = ones/l via tensor_tensor divide (keeps the in-order
       DVE queue free of sem-waiting ops so mixing streams ahead),
       scalar_tensor_tensor PSUM->oacc accumulate, small memsets, x-load
       SWDGE prep
  DMA  x f32->bf16 cast loads, DRAM bounce + 16x128-tile DMA transposes of
       the mixed Q/K naturals, per-query-block output stores

Schedule: configs processed h=12 -> h=8 -> h=4 so the first config's exp
stream (the longest) provides runway to mix/bounce/transpose everything
else behind it; per config passes run hf-outer/s-inner so half-1 operands
are needed as late as possible.  One software-pipelined attention stream
across all 3 configs (S^T+exp of job i, then PV of job i-1, crossing pass
and config boundaries); the driver pumps the next config's DVE mixing
between attention jobs.  x half-1 loads carry an explicit dep on the K
half-0 bounce so they don't cut ahead of the startup-critical transposes
on the FIFO DMA device.
"""

import math
from collections import deque

import numpy as np

import concourse.bass as bass
import concourse.bacc as bacc
import concourse.tile as tile
from concourse import mybir
from concourse.bass_utils import run_bass_kernel_spmd
from concourse.tile import add_dep_helper

F32 = mybir.dt.float32
BF16 = mybir.dt.bfloat16
ALU = mybir.AluOpType
ACTF = mybir.ActivationFunctionType

T = 1024
NT = 8  # token tiles of 128
E = 768
CIN = 3 * E
N_HEAD_LIST = (4, 8, 12)
CFG_ORDER = (2, 1, 0)  # process h=12 first: longest exp runway
N_CORES = 8
MASK_NEG = -3000.0  # additive pre-scale mask; exp(scale*MASK_NEG) == 0


def _pw(h):
    """Per-head column pitch in the natural mixed layout; h=8 pads 96 -> 128
    so every transposed head starts at a legal matmul base partition."""
    return 128 if h == 8 else E // h


def _dchunks(h):
    """Per head: contraction (d) row ranges in the transposed layout, split
    at 128-row tile boundaries."""
    hd = E // h
    pitch = _pw(h)
    out = []
    for i in range(h):
        a, b = i * pitch, i * pitch + hd
        chunks = []
        while a < b:
            nxt = min(b, (a // 128 + 1) * 128)
            chunks.append((a, nxt))
            a = nxt
        out.append(chunks)
    return out


def _build_program(W):
    """W: numpy [9] f32 mixture weights. Returns compiled Bacc program."""
    nc = bacc.Bacc(
        "TRN2", target_bir_lowering=False, debug=False, num_devices=N_CORES
    )
    x_in = nc.dram_tensor("x", [T, CIN], F32, kind="ExternalInput").ap()
    out_d = nc.dram_tensor("out", [T, E], F32, kind="ExternalOutput").ap()
    qk_dram = [
        [
            nc.dram_tensor(
                f"qkb_{ci}_{ti}", [T, N_HEAD_LIST[ci] * _pw(N_HEAD_LIST[ci])],
                BF16,
            ).ap()
            for ti in range(2)
        ]
        for ci in range(3)
    ]

    with tile.TileContext(nc) as tc:
        _emit(tc, x_in, out_d, qk_dram, W)
    nc.compile()
    return nc


def _emit(tc, x_in, out_d, qk_dram, W):
    nc = tc.nc
    with (
        tc.tile_pool(name="consts", bufs=1) as consts,
        tc.tile_pool(name="xbf", bufs=1) as xbf_pool,
        tc.tile_pool(name="nat", bufs=2) as nat_pool,
        tc.tile_pool(name="tmp", bufs=1) as tmp_pool,
        tc.tile_pool(name="qkt", bufs=2) as qkt_pool,
        tc.tile_pool(name="vaug", bufs=2) as vaug_pool,
        tc.tile_pool(name="pt", bufs=6) as pt_pool,
        tc.tile_pool(name="small", bufs=4) as small_pool,
        tc.tile_pool(name="oacc", bufs=1) as oacc_pool,
        tc.tile_pool(name="stage", bufs=2, space="PSUM") as stage_pool,
        tc.tile_pool(name="ypsum", bufs=4, space="PSUM") as ypsum_pool,
    ):
        xbf = xbf_pool.tile([128, NT, CIN], BF16)

        def load_x_chunk(third, half):
            c0 = third * E + half * (E // 2)
            return nc.gpsimd.dma_start(
                out=xbf[:, :, c0 : c0 + E // 2],
                in_=x_in[:, c0 : c0 + E // 2].rearrange(
                    "(a p) c -> p a c", p=128
                ),
            )

        # startup: q/k/v half-0 chunks first (q,k feed the critical mixes)
        load_x_chunk(0, 0)
        load_x_chunk(1, 0)
        load_x_chunk(2, 0)

        # ---- constants: strict-upper selector, MASK_NEG * I, ones row ---
        ustrict = consts.tile([128, 128], BF16)
        nc.gpsimd.memset(ustrict, 1.0)
        nc.gpsimd.affine_select(
            out=ustrict, in_=ustrict, compare_op=ALU.is_gt, fill=0.0,
            base=0, pattern=[[1, 128]], channel_multiplier=-1,
        )
        negi = consts.tile([128, 128], BF16)
        nc.gpsimd.memset(negi, 0.0)
        nc.gpsimd.affine_select(
            out=negi, in_=negi, compare_op=ALU.not_equal, fill=MASK_NEG,
            base=0, pattern=[[-1, 128]], channel_multiplier=1,
        )
        onesf = consts.tile([128, 8], F32)
        nc.gpsimd.memset(onesf, 1.0)
        ident = consts.tile([128, 128], BF16)
        nc.gpsimd.memset(ident, 0.0)
        nc.gpsimd.affine_select(
            out=ident, in_=ident, compare_op=ALU.not_equal, fill=1.0,
            base=0, pattern=[[-1, 128]], channel_multiplier=1,
        )
        # dummy exp: hoists the ACT table load off the first real exp's path
        scratch = consts.tile([128, 8], F32)
        nc.scalar.activation(
            out=scratch[:, 0:1], in_=onesf[:, 0:1], func=ACTF.Exp, scale=1.0
        )

        oacc = oacc_pool.tile([128, NT, E], F32)

        state = {}

        # weight order in W: for cfg ci, e in (384, 576, 768): W[3*ci + idx]
        def mix_config(oi):
            """Generator. Emits DVE mixing + bounce/transpose DMAs for one
            config (order index oi), yielding between DVE ops.  Yields
            "ready" (oi==0 only) once attention may start."""
            ci = CFG_ORDER[oi]
            h = N_HEAD_LIST[ci]
            hd = E // h
            pw = _pw(h)
            h2 = h // 2
            e_list = [(2, 768, hd), (1, 576, 576 // h), (0, 384, 384 // h)]
            ndt = h * pw // 128
            ndt2 = ndt // 2

            qkt = []
            vaug = vaug_pool.tile([128, NT, h, hd + 1], BF16, tag="vaug")
            for tensor_idx in range(2):
                tl = qkt_pool.tile(
                    [128, ndt, T], BF16, tag="qkt", bufs=4,
                    name=f"qkt{ci}{tensor_idx}",
                )
                qkt.append(tl)
            tmp = tmp_pool.tile([128, NT, 288], BF16, tag="tmp")
            tmpb = tmp_pool.tile([128, NT, 288], BF16, tag="tmpb")
            vtmp = tmp_pool.tile([128, NT, 288], BF16, tag="vtmp")
            vtmpb = tmp_pool.tile([128, NT, 288], BF16, tag="vtmpb")
            state[ci] = (qkt, vaug)

            def mix_into(out_ap, xsrc, tmps, eng=None):
                """Yields after each mixing op. out_ap(hde) is the dest
                slice, xsrc(e, hde) the source slice for mixture term e."""
                eng = eng or nc.vector
                for idx, (k, e, hde) in enumerate(e_list):
                    w = float(W[3 * ci + k])
                    in0 = xsrc(e, hde)
                    if idx == 0:
                        yield eng.tensor_scalar(
                            out_ap(hde), in0, w, None, ALU.mult
                        )
                    else:
                        tview = tmps[idx % len(tmps)].rearrange(
                            "p a (h d) -> p a h d", h=h2
                        )
                        tv = tview[:, :, :, 0:hde]
                        yield eng.tensor_scalar(
                            tv, in0, w, None, ALU.mult
                        )
                        yield eng.tensor_tensor(
                            out_ap(hde), tv, out_ap(hde), ALU.add
                        )

            for half in range(2):
                if oi == 0 and half == 1:
                    for third in range(3):
                        load_x_chunk(third, 1)
                hsl = slice(half * h2, (half + 1) * h2)
                for tensor_idx in range(2):
                    base = tensor_idx * E
                    nat = nat_pool.tile([128, NT, h2, pw], BF16, tag="nat")
                    if pw > hd:
                        nc.vector.memset(nat[:, :, :, hd:pw], 0.0)

                    def xsrc(e, hde, base=base, half=half):
                        sl = xbf[
                            :, :,
                            base + half * (e // 2)
                            : base + (half + 1) * (e // 2),
                        ]
                        return sl.rearrange("p a (h d) -> p a h d", h=h2)

                    def out_ap(hde, nat=nat):
                        return nat[:, :, :, 0:hde]

                    for _ in mix_into(out_ap, xsrc, (tmp, tmpb)):
                        yield

                    if False and oi == 0 and half == 0:
                        # startup path: PE-transpose through PSUM (no DRAM
                        # bounce -- HWDGE DMAs serialize ring-to-ring and
                        # would push the first exp out by ~15us).  Stage
                        # PSUM is still unused this early; first transpose
                        # into each 2KB bank carries start=True.
                        natflat = nat.rearrange("p a h d -> p a (h d)")
                        tps = []
                        for c in range(ndt2):
                            bank = c % 2
                            if bank == 0:
                                tp = stage_pool.tile(
                                    [128, 2, T], BF16, tag="stage"
                                )
                                tps.append(tp)
                            first = None
                            for a in range(NT):
                                mm = nc.tensor.matmul(
                                    tp[:, bank, a * 128 : (a + 1) * 128],
                                    natflat[:, a, c * 128 : (c + 1) * 128],
                                    ident,
                                    is_transpose=True,
                                    start=(a == 0),
                                    stop=True,
                                    skip_group_check=True,
                                )
                                if first is None:
                                    first = mm
                                else:
                                    add_dep_helper(
                                        mm.ins, first.ins,
                                        reason="psum zero-region order",
                                    )
                            yield
                        for c in range(ndt2):
                            nc.vector.tensor_copy(
                                qkt[tensor_idx][:, c, :],
                                tps[c // 2][:, c % 2, :],
                            )
                            yield
                        if tensor_idx == 1:
                            yield "ready"
                        continue

                    # steady state: bounce to DRAM + one 3D transpose read
                    # on the SP ring
                    eng = nc.sync
                    w0 = half * h2 * pw
                    wr = eng.dma_start(
                        out=qk_dram[ci][tensor_idx][
                            :, w0 : w0 + h2 * pw
                        ].rearrange("(a p) w -> p a w", p=128),
                        in_=nat[:, :, :, :],
                    )
                    rd = eng.dma_start(
                        out=qkt[tensor_idx][
                            :, half * ndt2 : (half + 1) * ndt2, :
                        ],
                        in_=qk_dram[ci][tensor_idx][:, w0 : w0 + h2 * pw],
                        transpose=True,
                    )
                    add_dep_helper(
                        rd.ins, wr.ins, sync=True, reason="dram bounce raw"
                    )
                    if oi == 0 and half == 0 and tensor_idx == 1:
                        yield "ready"
                    yield

                # V_aug for this half
                nc.gpsimd.memset(vaug[:, :, hsl, hd : hd + 1], 1.0)

                def vsrc(e, hde, half=half):
                    sl = xbf[
                        :, :,
                        2 * E + half * (e // 2)
                        : 2 * E + (half + 1) * (e // 2),
                    ]
                    return sl.rearrange("p a (h d) -> p a h d", h=h2)

                def vout(hde, hsl=hsl):
                    return vaug[:, :, hsl, 0:hde]

                # V mixing runs on Pool (idle capacity; SBUF-only ops) with
                # its own scratch, except the startup half which must be
                # ready fast -- that one stays on the DVE
                if oi == 0 and half == 0:
                    for _ in mix_into(vout, vsrc, (tmp, tmpb)):
                        yield
                else:
                    for _ in mix_into(vout, vsrc, (vtmp, vtmpb),
                                      eng=nc.gpsimd):
                        yield

        def attention():
            """Single software-pipelined job stream across all 3 configs."""
            prev = [None]  # carried (emit_fn, tk, g, ptl) across passes

            for oi, ci in enumerate(CFG_ORDER):
                if oi > 0:
                    yield ("cfg", oi)
                h = N_HEAD_LIST[ci]
                hd = E // h
                h2 = h // 2
                scale = 1.0 / math.sqrt(hd)
                dchunks = _dchunks(h)
                qkt, vaug = state[ci]
                qt, kt = qkt

                for s in range(2):
                    for hf in range(2):
                        ntk = 4 * s + 4
                        pheads = list(range(hf * h2, (hf + 1) * h2))
                        nh = h2
                        groups = [
                            pheads[i : i + 2] for i in range(0, nh, 2)
                        ]
                        yts = [
                            ypsum_pool.tile(
                                [128, nh, hd + 1], F32, tag="y",
                                name=f"yt{ci}{s}{hf}{qt_}",
                            )
                            for qt_ in range(4)
                        ]
                        y_first = [None] * 4

                        def norm_qt(qt_, *, oi=oi, s=s, hf=hf, yts=yts,
                                    pheads=pheads, nh=nh, hd=hd):
                            tqg = 4 * s + qt_
                            c0 = pheads[0] * hd
                            dst = oacc[
                                :, tqg, c0 : c0 + nh * hd
                            ].rearrange("p (h d) -> p h d", h=nh)
                            # lrow copy to SBUF, then rec = 1/l; TT divide
                            # is not in the DVE ALU op set and reciprocal
                            # needs an SBUF source on hardware
                            lrow = small_pool.tile(
                                [128, 8], F32, tag="lrow", bufs=4
                            )
                            rec = small_pool.tile(
                                [128, 8], F32, tag="rec", bufs=4
                            )
                            nc.vector.tensor_copy(
                                lrow[:, 0:nh], yts[qt_][:, :, hd]
                            )
                            nc.vector.reciprocal(
                                rec[:, 0:nh], lrow[:, 0:nh]
                            )
                            for jp in range(nh):
                                dj = dst[:, jp, :]
                                if oi == 0:
                                    nc.vector.tensor_scalar(
                                        dj, yts[qt_][:, jp, 0:hd],
                                        rec[:, jp : jp + 1], None, ALU.mult,
                                    )
                                else:
                                    nc.vector.scalar_tensor_tensor(
                                        out=dj,
                                        in0=yts[qt_][:, jp, 0:hd],
                                        scalar=rec[:, jp : jp + 1],
                                        in1=dj,
                                        op0=ALU.mult,
                                        op1=ALU.add,
                                    )
                            if oi == len(CFG_ORDER) - 1 and hf == 1:
                                # this query tile is final: stream out
                                nc.sync.dma_start(
                                    out=out_d[tqg * 128 : (tqg + 1) * 128, :],
                                    in_=oacc[:, tqg, :],
                                )

                        pending = deque()

                        def emit_pv(tk, g, ptl, *, s=s, hf=hf, nh=nh, hd=hd,
                                    yts=yts, y_first=y_first, vaug=vaug,
                                    groups=groups, norm_qt=norm_qt,
                                    pending=pending, ntk=ntk):
                            for qt_ in range(4):
                                qtg = 4 * s + qt_
                                if qtg < tk:
                                    continue
                                for j, head in enumerate(g):
                                    jp = head - hf * nh
                                    is_start = (
                                        tk == 0 and y_first[qt_] is None
                                    )
                                    mm = nc.tensor.matmul(
                                        out=yts[qt_][:, jp, :],
                                        lhsT=ptl[
                                            :, j, qt_ * 128 : (qt_ + 1) * 128
                                        ],
                                        rhs=vaug[:, tk, head, :],
                                        start=is_start,
                                        stop=(tk == qtg and jp == nh - 1),
                                    )
                                    if is_start:
                                        y_first[qt_] = mm
                                    elif tk == 0:
                                        add_dep_helper(
                                            mm.ins,
                                            y_first[qt_].ins,
                                            reason="psum zero-region order",
                                        )
                            # norms: delay emission ~2 jobs so their PV-stop
                            # waits are pre-satisfied and never throttle the
                            # in-order DVE queue; flush at pass end
                            if g is groups[-1] and 0 <= tk - 4 * s < 4:
                                pending.append(tk - 4 * s)
                            while len(pending) > 0:
                                norm_qt(pending.popleft())

                        for tk in range(ntk):
                            lo = max(0, tk * 128 - s * 512)
                            diag = tk >= 4 * s
                            dlo = tk * 128 - s * 512
                            for g in groups:
                                stage = stage_pool.tile(
                                    [128, 2, 512], F32, tag="stage"
                                )
                                for j, head in enumerate(g):
                                    chunks = dchunks[head]
                                    n_mm = len(chunks) + (1 if diag else 0)
                                    for mi, (a, b) in enumerate(chunks):
                                        nc.tensor.matmul(
                                            out=stage[:, j, lo:512],
                                            lhsT=kt[
                                                a % 128 : a % 128 + (b - a),
                                                a // 128,
                                                tk * 128 : (tk + 1) * 128,
                                            ],
                                            rhs=qt[
                                                a % 128 : a % 128 + (b - a),
                                                a // 128,
                                                s * 512 + lo : (s + 1) * 512,
                                            ],
                                            start=(mi == 0),
                                            stop=(mi == n_mm - 1),
                                        )
                                    if diag:
                                        nc.tensor.matmul(
                                            out=stage[:, j, dlo : dlo + 128],
                                            lhsT=ustrict[:, :],
                                            rhs=negi[:, :],
                                            start=False,
                                            stop=True,
                                        )
                                ptl = pt_pool.tile(
                                    [128, 2, 512], BF16, tag="pt"
                                )
                                nc.scalar.activation(
                                    out=ptl[:, 0:2, lo:512],
                                    in_=stage[:, 0:2, lo:512],
                                    func=ACTF.Exp,
                                    scale=scale,
                                )
                                if prev[0] is not None:
                                    pfn, ptk, pg, pptl = prev[0]
                                    pfn(ptk, pg, pptl)
                                prev[0] = (emit_pv, tk, g, ptl)
                                yield
                        if prev[0] is not None:
                            pfn, ptk, pg, pptl = prev[0]
                            pfn(ptk, pg, pptl)
                            prev[0] = None
            if prev[0] is not None:
                pfn, ptk, pg, pptl = prev[0]
                pfn(ptk, pg, pptl)

        # ---- driver: startup mix, then attention with mix pumping ------
        gens = deque([(oi, mix_config(oi)) for oi in range(3)])
        g0 = gens[0][1]
        while True:
            if next(g0) == "ready":
                break

        def pump(n):
            for _ in range(n):
                while gens:
                    try:
                        next(gens[0][1])
                        break
                    except StopIteration:
                        gens.popleft()
                else:
                    return

        def drain_through(oi):
            while gens and gens[0][0] <= oi:
                try:
                    next(gens[0][1])
                except StopIteration:
                    gens.popleft()

        for item in attention():
            if isinstance(item, tuple) and item[0] == "cfg":
                drain_through(item[1])
            else:
                pump(2)
        while gens:
            try:
                next(gens[0][1])
            except StopIteration:
                gens.popleft()


_PROGRAM_CACHE = {}


def _get_program(W):
    key = np.asarray(W, dtype=np.float32).tobytes()
    if key not in _PROGRAM_CACHE:
        _PROGRAM_CACHE[key] = _build_program(np.asarray(W, dtype=np.float32))
    return _PROGRAM_CACHE[key]


def kernel(x, weights):
    """x: [8, 1024, 2304] f32; weights: [9] f32 -> [8, 1024, 768] f32."""
    x = np.asarray(x, dtype=np.float32)
    weights = np.asarray(weights, dtype=np.float32)
    assert x.shape == (N_CORES, T, CIN), x.shape
    nc = _get_program(weights)
    in_maps = [{"x": np.ascontiguousarray(x[c])} for c in range(N_CORES)]
    res = run_bass_kernel_spmd(nc, in_maps, list(range(N_CORES)))
    return np.stack([res.results[c]["out"] for c in range(N_CORES)], axis=0)


# revision 41
# speedup vs baseline: 1.0935x; 1.0935x over previous
"""Trainium2 Bass kernel for nn_MixedAttnHeadEmbed_82076825027210.

Computes, per batch element:
    out = sum over h in {4, 8, 12} of CausalAttention(Q_mix_h, K_mix_h, V_mix_h)
where Q/K/V_mix_h are weighted mixtures (9 scalar weights) of head-sliced
views of x's q/k/v channel groups, padded per head to hd = 768/h.

Sharding: data-parallel over batch B=8 across the 8 NeuronCores (one batch
element per core); the 9 mixture weights are baked into the compiled program
as immediates.

Per-core plan (T=1024 tokens, bf16 compute, fp32 accumulation):
  1. Six SWDGE cast-DMAs load x [1024, 2304] f32 -> SBUF bf16 in half-head
     column chunks so mixing starts as soon as the first chunk lands.
  2. Per config and per half of the heads, DVE builds mixed Q/K naturals
     (tensor_scalar at 4x + tensor_tensor adds at 2x -- scalar_tensor_tensor
     runs at 1x) and V_aug with a ones column per head for the softmax
     denominator.
  3. Each half bounces through DRAM and returns via HWDGE DMA-transpose as
     Q^T/K^T [d, T] bf16 matmul operands, all on the SP ring (one ring keeps
     the DRAM RAW ordering real; splitting across the ACT ring raced), so
     attention pass 0 starts after half of config 0's mixing.
  4. Attention per config, per 512-query block, per half-of-heads pass:
     S^T = K_mix Q_mix^T blockwise on PE (causal blocks only; diagonal
     blocks masked by one extra ustrict x negi matmul per head), exp on ACT
     with the softmax scale folded in (max-subtraction skipped: |S*scale|
     is small), then Y = P V_aug accumulated *natural* (queries on
     partitions) in PSUM with P^T tiles as the stationary operand -- the
     ones-column lands the denominator l as an extra output column. PSUM
     start=True marks a whole 2KB zero region, so only the first matmul
     into each Y bank carries it (with explicit ordering deps).
  5. Per query tile, the moment its accumulation stops: DVE reciprocal of
     l, then scalar_tensor_tensor normalize-and-accumulate from PSUM into
     the fp32 output accumulator; results stream out in per-query-block
     SWDGE DMAs as the last config finishes. Emission is driven through
     generators so each config's DVE mixing interleaves with the previous
     config's attention normalizes (engines execute in emission order).
"""

import math

import numpy as np

import concourse.bass as bass
import concourse.bacc as bacc
import concourse.tile as tile
from concourse import mybir
from concourse.bass_utils import run_bass_kernel_spmd
from concourse.tile import add_dep_helper

F32 = mybir.dt.float32
BF16 = mybir.dt.bfloat16
ALU = mybir.AluOpType
ACTF = mybir.ActivationFunctionType

T = 1024
NT = 8  # token tiles of 128
E = 768
CIN = 3 * E
N_HEAD_LIST = (4, 8, 12)
N_CORES = 8
MASK_NEG = -3000.0  # additive pre-scale mask; exp(scale*MASK_NEG) == 0


def _pw(h):
    """Per-head column pitch in the natural mixed layout; h=8 pads 96 -> 128
    so every transposed head starts at a legal matmul base partition."""
    return 128 if h == 8 else E // h


def _dchunks(h):
    """Per head: contraction (d) row ranges in the transposed layout, split
    at 128-row tile boundaries."""
    hd = E // h
    pitch = _pw(h)
    out = []
    for i in range(h):
        a, b = i * pitch, i * pitch + hd
        chunks = []
        while a < b:
            nxt = min(b, (a // 128 + 1) * 128)
            chunks.append((a, nxt))
            a = nxt
        out.append(chunks)
    return out


def _build_program(W):
    """W: numpy [9] f32 mixture weights. Returns compiled Bacc program."""
    nc = bacc.Bacc(
        "TRN2", target_bir_lowering=False, debug=False, num_devices=N_CORES
    )
    x_in = nc.dram_tensor("x", [T, CIN], F32, kind="ExternalInput").ap()
    out_d = nc.dram_tensor("out", [T, E], F32, kind="ExternalOutput").ap()
    qk_dram = [
        [
            nc.dram_tensor(
                f"qkb_{ci}_{ti}", [T, N_HEAD_LIST[ci] * _pw(N_HEAD_LIST[ci])],
                BF16,
            ).ap()
            for ti in range(2)
        ]
        for ci in range(3)
    ]

    with tile.TileContext(nc) as tc:
        _emit(tc, x_in, out_d, qk_dram, W)
    nc.compile()
    return nc


def _mix_half(nc, W, ci, out_ap, xsrc, tmps, e_list, h2, add_eng=None):
    """Mixed half-tensor: out[:, :, i, 0:e/h] (+)= w_e * xsrc_e per e.
    tensor_scalar (4x) for the largest e, then ts into tmp + tensor_tensor
    add (2x) for the rest -- scalar_tensor_tensor would run at 1x. The adds
    can run on POOL (add_eng) to offload the DVE."""
    add_eng = add_eng or nc.vector
    for idx, (k, e, hde) in enumerate(e_list):
        w = float(W[3 * ci + k])
        in0 = xsrc(e, hde)
        if idx == 0:
            nc.vector.tensor_scalar(
                out_ap(hde), in0, w, None, ALU.mult
            )
        else:
            tview = tmps[idx % len(tmps)].rearrange(
                "p a (h d) -> p a h d", h=h2
            )
            tv = tview[:, :, :, 0:hde]
            nc.vector.tensor_scalar(tv, in0, w, None, ALU.mult)
            add_eng.tensor_tensor(out_ap(hde), tv, out_ap(hde), ALU.add)


def _emit(tc, x_in, out_d, qk_dram, W):
    nc = tc.nc
    with (
        tc.tile_pool(name="consts", bufs=1) as consts,
        tc.tile_pool(name="xbf", bufs=1) as xbf_pool,
        tc.tile_pool(name="nat", bufs=2) as nat_pool,
        tc.tile_pool(name="tmp", bufs=1) as tmp_pool,
        tc.tile_pool(name="qkt", bufs=2) as qkt_pool,
        tc.tile_pool(name="vaug", bufs=3) as vaug_pool,
        tc.tile_pool(name="pt", bufs=6) as pt_pool,
        tc.tile_pool(name="small", bufs=4) as small_pool,
        tc.tile_pool(name="oacc", bufs=1) as oacc_pool,
        tc.tile_pool(name="stage", bufs=2, space="PSUM") as stage_pool,
        tc.tile_pool(name="ypsum", bufs=4, space="PSUM") as ypsum_pool,
    ):
        # ---- constants: strict-upper selector and MASK_NEG * I ----------
        ustrict = consts.tile([128, 128], BF16)
        nc.gpsimd.memset(ustrict, 1.0)
        nc.gpsimd.affine_select(
            out=ustrict, in_=ustrict, compare_op=ALU.is_gt, fill=0.0,
            base=0, pattern=[[1, 128]], channel_multiplier=-1,
        )
        negi = consts.tile([128, 128], BF16)
        nc.gpsimd.memset(negi, 0.0)
        nc.gpsimd.affine_select(
            out=negi, in_=negi, compare_op=ALU.not_equal, fill=MASK_NEG,
            base=0, pattern=[[-1, 128]], channel_multiplier=1,
        )

        # ---- x loads happen in half-head column chunks, cast to bf16;
        # half-1 chunks are emitted mid-way through config 0's mixing so
        # config 0's bounce DMAs are not queued behind the whole 9.4MB ----
        xbf = xbf_pool.tile([128, NT, CIN], BF16)

        def load_x_half(half):
            for third in range(3):
                c0 = third * E + half * (E // 2)
                nc.gpsimd.dma_start(
                    out=xbf[:, :, c0 : c0 + E // 2],
                    in_=x_in[:, c0 : c0 + E // 2].rearrange(
                        "(a p) c -> p a c", p=128
                    ),
                )

        oacc = oacc_pool.tile([128, NT, E], F32)

        state = {}

        # weight order in W: for cfg ci, e in (384, 576, 768): W[3*ci + idx]
        def mix_config(ci):
            # generator: yields after each (half, tensor) piece so the
            # driver can interleave DVE mixing with the previous config's
            # attention normalizes (DVE executes in emission order)
            h = N_HEAD_LIST[ci]
            hd = E // h
            pw = _pw(h)
            h2 = h // 2
            scale = 1.0 / math.sqrt(hd)
            dchunks = _dchunks(h)
            ndt = h * pw // 128
            ndt2 = ndt // 2
            e_list = [(2, 768, hd), (1, 576, 576 // h), (0, 384, 384 // h)]

            # ---- mix + bounce + transpose per half of the heads ---------
            qkt = []
            vaug = vaug_pool.tile([128, NT, h, hd + 1], BF16, tag="vaug")
            for tensor_idx in range(2):
                tl = qkt_pool.tile(
                    [128, ndt, T], BF16, tag="qkt", bufs=4,
                    name=f"qkt{ci}{tensor_idx}",
                )
                qkt.append(tl)
            tmp = tmp_pool.tile([128, NT, 288], BF16, tag="tmp")
            tmpb = tmp_pool.tile([128, NT, 288], BF16, tag="tmpb")
            state[ci] = (qkt, vaug)
            for half in range(2):
                if ci == 0:
                    load_x_half(half)
                hsl = slice(half * h2, (half + 1) * h2)
                for tensor_idx in range(2):  # 0=Q (SP ring) 1=K (ACT ring)
                    base = tensor_idx * E
                    nat = nat_pool.tile(
                        [128, NT, h2, pw], BF16, tag="nat"
                    )
                    if pw > hd:
                        nc.vector.memset(nat[:, :, :, hd:pw], 0.0)

                    def xsrc(e, hde, base=base, half=half):
                        sl = xbf[
                            :, :,
                            base + half * (e // 2) : base + (half + 1) * (e // 2),
                        ]
                        return sl.rearrange("p a (h d) -> p a h d", h=h2)

                    def out_ap(hde, nat=nat):
                        return nat[:, :, :, 0:hde]

                    _mix_half(nc, W, ci, out_ap, xsrc, (tmp, tmpb), e_list, h2)

                    eng = nc.sync  # single HWDGE ring (dual-ring raced)
                    w0 = half * h2 * pw
                    wr = eng.dma_start(
                        out=qk_dram[ci][tensor_idx][
                            :, w0 : w0 + h2 * pw
                        ].rearrange("(a p) w -> p a w", p=128),
                        in_=nat[:, :, :, :],
                    )
                    for dt_ in range(half * ndt2, (half + 1) * ndt2):
                        rd = eng.dma_start(
                            out=qkt[tensor_idx][:, dt_, :],
                            in_=qk_dram[ci][tensor_idx][
                                :, dt_ * 128 : (dt_ + 1) * 128
                            ],
                            transpose=True,
                        )
                        add_dep_helper(
                            rd.ins, wr.ins, sync=True,
                            reason="dram bounce raw",
                        )
                    yield

                # V_aug for this half
                nc.vector.memset(vaug[:, :, hsl, hd : hd + 1], 1.0)

                def vsrc(e, hde, half=half):
                    sl = xbf[
                        :, :,
                        2 * E + half * (e // 2) : 2 * E + (half + 1) * (e // 2),
                    ]
                    return sl.rearrange("p a (h d) -> p a h d", h=h2)

                def vout(hde, hsl=hsl):
                    return vaug[:, :, hsl, 0:hde]

                _mix_half(nc, W, ci, vout, vsrc, (tmp, tmpb), e_list, h2,
                           add_eng=nc.gpsimd)
                yield

        def attn_config(ci):
            h = N_HEAD_LIST[ci]
            hd = E // h
            h2 = h // 2
            scale = 1.0 / math.sqrt(hd)
            dchunks = _dchunks(h)
            qkt, vaug = state.pop(ci)
            qt, kt = qkt

            # ---- attention ---------------------------------------------
            for s in range(2):
                ntk = 4 * s + 4
                for hf in range(2):
                    pheads = list(range(hf * h2, (hf + 1) * h2))
                    nh = h2
                    groups = [pheads[i : i + 2] for i in range(0, nh, 2)]
                    yts = [
                        ypsum_pool.tile(
                            [128, nh, hd + 1], F32, tag="y", name=f"yt{qt_}"
                        )
                        for qt_ in range(4)
                    ]
                    # One accumulation start per PSUM bank: start=True marks
                    # the whole 2KB zero region pending-zero, so only the
                    # first matmul in each Y bank carries it; later heads'
                    # first writes overwrite via the pending-zero bytes.
                    y_first = [None] * 4

                    def norm_qt(qt_):
                        tqg = 4 * s + qt_
                        lrow = small_pool.tile([128, 6], F32, tag="lrow")
                        rec = small_pool.tile([128, 6], F32, tag="rec")
                        nc.vector.tensor_copy(
                            lrow[:, 0:nh], yts[qt_][:, :, hd]
                        )
                        nc.vector.reciprocal(rec[:, 0:nh], lrow[:, 0:nh])
                        for jp, head in enumerate(pheads):
                            dst = oacc[:, tqg, head * hd : head * hd + hd]
                            if ci == 0:
                                nc.vector.tensor_scalar(
                                    dst, yts[qt_][:, jp, 0:hd],
                                    rec[:, jp : jp + 1], None, ALU.mult,
                                )
                            else:
                                nc.vector.scalar_tensor_tensor(
                                    out=dst,
                                    in0=yts[qt_][:, jp, 0:hd],
                                    scalar=rec[:, jp : jp + 1],
                                    in1=dst,
                                    op0=ALU.mult,
                                    op1=ALU.add,
                                )
                        if ci == 2 and hf == 1:
                            # this query tile is final: stream out
                            nc.gpsimd.dma_start(
                                out=out_d[tqg * 128 : (tqg + 1) * 128, :],
                                in_=oacc[:, tqg, :],
                            )

                    def emit_pv(tk, g, ptl):
                        for qt_ in range(4):
                            qtg = 4 * s + qt_
                            if qtg < tk:
                                continue
                            for j, head in enumerate(g):
                                jp = head - hf * h2
                                is_start = tk == 0 and y_first[qt_] is None
                                mm = nc.tensor.matmul(
                                    out=yts[qt_][:, jp, :],
                                    lhsT=ptl[
                                        :, j, qt_ * 128 : (qt_ + 1) * 128
                                    ],
                                    rhs=vaug[:, tk, head, :],
                                    start=is_start,
                                    stop=(tk == qtg and jp == nh - 1),
                                )
                                if is_start:
                                    y_first[qt_] = mm
                                elif tk == 0:
                                    add_dep_helper(
                                        mm.ins,
                                        y_first[qt_].ins,
                                        reason="psum zero-region order",
                                    )
                        if g is groups[-1] and 0 <= tk - 4 * s < 4:
                            norm_qt(tk - 4 * s)

                    # software pipeline: emit S^T+exp for job i, then the
                    # PV matmuls of job i-1, so the in-order PE stream is
                    # never head-of-line blocked on the current exp
                    prev = None
                    for tk in range(ntk):
                        lo = max(0, tk * 128 - s * 512)
                        diag = tk >= 4 * s
                        dlo = tk * 128 - s * 512
                        for g in groups:
                            stage = stage_pool.tile(
                                [128, 2, 512], F32, tag="stage"
                            )
                            for j, head in enumerate(g):
                                chunks = dchunks[head]
                                n_mm = len(chunks) + (1 if diag else 0)
                                for mi, (a, b) in enumerate(chunks):
                                    nc.tensor.matmul(
                                        out=stage[:, j, lo:512],
                                        lhsT=kt[
                                            a % 128 : a % 128 + (b - a),
                                            a // 128,
                                            tk * 128 : (tk + 1) * 128,
                                        ],
                                        rhs=qt[
                                            a % 128 : a % 128 + (b - a),
                                            a // 128,
                                            s * 512 + lo : (s + 1) * 512,
                                        ],
                                        start=(mi == 0),
                                        stop=(mi == n_mm - 1),
                                    )
                                if diag:
                                    nc.tensor.matmul(
                                        out=stage[:, j, dlo : dlo + 128],
                                        lhsT=ustrict[:, :],
                                        rhs=negi[:, :],
                                        start=False,
                                        stop=True,
                                    )
                            ptl = pt_pool.tile([128, 2, 512], BF16, tag="pt")
                            nc.scalar.activation(
                                out=ptl[:, 0:2, lo:512],
                                in_=stage[:, 0:2, lo:512],
                                func=ACTF.Exp,
                                scale=scale,
                            )
                            if prev is not None:
                                emit_pv(*prev)
                            prev = (tk, g, ptl)
                    if prev is not None:
                        emit_pv(*prev)
                    yield

        def adv(gen, n=1):
            if gen is None:
                return None
            for _ in range(n):
                try:
                    next(gen)
                except StopIteration:
                    return None
            return gen

        for _ in mix_config(0):
            pass
        m_next = mix_config(1)
        for ci in range(3):
            a = attn_config(ci)
            while True:
                try:
                    next(a)
                except StopIteration:
                    break
                m_next = adv(m_next, 3)
            if m_next is not None:
                for _ in m_next:
                    pass
            m_next = mix_config(2) if ci == 0 else None


_PROGRAM_CACHE = {}


def _get_program(W):
    key = np.asarray(W, dtype=np.float32).tobytes()
    if key not in _PROGRAM_CACHE:
        _PROGRAM_CACHE[key] = _build_program(np.asarray(W, dtype=np.float32))
    return _PROGRAM_CACHE[key]


def kernel(x, weights):
    """x: [8, 1024, 2304] f32; weights: [9] f32 -> [8, 1024, 768] f32."""
    x = np.asarray(x, dtype=np.float32)
    weights = np.asarray(weights, dtype=np.float32)
    assert x.shape == (N_CORES, T, CIN), x.shape
    nc = _get_program(weights)
    in_maps = [{"x": np.ascontiguousarray(x[c])} for c in range(N_CORES)]
    res = run_bass_kernel_spmd(nc, in_maps, list(range(N_CORES)))
    return np.stack([res.results[c]["out"] for c in range(N_CORES)], axis=0)



# revision 42
# speedup vs baseline: 1.1024x; 1.0082x over previous
"""Trainium2 Bass kernel for nn_MixedAttnHeadEmbed_82076825027210.

Computes, per batch element:
    out = sum over h in {4, 8, 12} of CausalAttention(Q_mix_h, K_mix_h, V_mix_h)
where Q/K/V_mix_h are weighted mixtures (9 scalar weights) of head-sliced
views of x's q/k/v channel groups, padded per head to hd = 768/h.

Sharding: data-parallel over batch B=8 across the 8 NeuronCores (one batch
element per core); the 9 mixture weights are baked into the compiled program
as immediates.

Per-core plan (T=1024 tokens, bf16 compute, fp32 accumulation):
  1. Six SWDGE cast-DMAs load x [1024, 2304] f32 -> SBUF bf16 in half-head
     column chunks so mixing starts as soon as the first chunk lands.
  2. Per config and per half of the heads, DVE builds mixed Q/K naturals
     (tensor_scalar at 4x + tensor_tensor adds at 2x -- scalar_tensor_tensor
     runs at 1x) and V_aug with a ones column per head for the softmax
     denominator.
  3. Each half bounces through DRAM and returns via HWDGE DMA-transpose as
     Q^T/K^T [d, T] bf16 matmul operands, all on the SP ring (one ring keeps
     the DRAM RAW ordering real; splitting across the ACT ring raced), so
     attention pass 0 starts after half of config 0's mixing.
  4. Attention per config, per 512-query block, per half-of-heads pass:
     S^T = K_mix Q_mix^T blockwise on PE (causal blocks only; diagonal
     blocks masked by one extra ustrict x negi matmul per head), exp on ACT
     with the softmax scale folded in (max-subtraction skipped: |S*scale|
     is small), then Y = P V_aug accumulated *natural* (queries on
     partitions) in PSUM with P^T tiles as the stationary operand -- the
     ones-column lands the denominator l as an extra output column. PSUM
     start=True marks a whole 2KB zero region, so only the first matmul
     into each Y bank carries it (with explicit ordering deps).
  5. Per query tile, the moment its accumulation stops: DVE reciprocal of
     l, then scalar_tensor_tensor normalize-and-accumulate from PSUM into
     the fp32 output accumulator; results stream out in per-query-block
     SWDGE DMAs as the last config finishes. Emission is driven through
     generators so each config's DVE mixing interleaves with the previous
     config's attention normalizes (engines execute in emission order).
"""

import math

import numpy as np

import concourse.bass as bass
import concourse.bacc as bacc
import concourse.tile as tile
from concourse import mybir
from concourse.bass_utils import run_bass_kernel_spmd
from concourse.tile import add_dep_helper

F32 = mybir.dt.float32
BF16 = mybir.dt.bfloat16
ALU = mybir.AluOpType
ACTF = mybir.ActivationFunctionType

T = 1024
NT = 8  # token tiles of 128
E = 768
CIN = 3 * E
N_HEAD_LIST = (4, 8, 12)
N_CORES = 8
MASK_NEG = -3000.0  # additive pre-scale mask; exp(scale*MASK_NEG) == 0


def _pw(h):
    """Per-head column pitch in the natural mixed layout; h=8 pads 96 -> 128
    so every transposed head starts at a legal matmul base partition."""
    return 128 if h == 8 else E // h


def _dchunks(h):
    """Per head: contraction (d) row ranges in the transposed layout, split
    at 128-row tile boundaries."""
    hd = E // h
    pitch = _pw(h)
    out = []
    for i in range(h):
        a, b = i * pitch, i * pitch + hd
        chunks = []
        while a < b:
            nxt = min(b, (a // 128 + 1) * 128)
            chunks.append((a, nxt))
            a = nxt
        out.append(chunks)
    return out


def _build_program(W):
    """W: numpy [9] f32 mixture weights. Returns compiled Bacc program."""
    nc = bacc.Bacc(
        "TRN2", target_bir_lowering=False, debug=False, num_devices=N_CORES
    )
    x_in = nc.dram_tensor("x", [T, CIN], F32, kind="ExternalInput").ap()
    out_d = nc.dram_tensor("out", [T, E], F32, kind="ExternalOutput").ap()
    qk_dram = [
        [
            nc.dram_tensor(
                f"qkb_{ci}_{ti}", [T, N_HEAD_LIST[ci] * _pw(N_HEAD_LIST[ci])],
                BF16,
            ).ap()
            for ti in range(2)
        ]
        for ci in range(3)
    ]

    with tile.TileContext(nc) as tc:
        _emit(tc, x_in, out_d, qk_dram, W)
    nc.compile()
    return nc


def _mix_half(nc, W, ci, out_ap, xsrc, tmps, e_list, h2, add_eng=None):
    """Mixed half-tensor: out[:, :, i, 0:e/h] (+)= w_e * xsrc_e per e.
    tensor_scalar (4x) for the largest e, then ts into tmp + tensor_tensor
    add (2x) for the rest -- scalar_tensor_tensor would run at 1x. The adds
    can run on POOL (add_eng) to offload the DVE."""
    add_eng = add_eng or nc.vector
    for idx, (k, e, hde) in enumerate(e_list):
        w = float(W[3 * ci + k])
        in0 = xsrc(e, hde)
        if idx == 0:
            nc.vector.tensor_scalar(
                out_ap(hde), in0, w, None, ALU.mult
            )
        else:
            tview = tmps[idx % len(tmps)].rearrange(
                "p a (h d) -> p a h d", h=h2
            )
            tv = tview[:, :, :, 0:hde]
            nc.vector.tensor_scalar(tv, in0, w, None, ALU.mult)
            add_eng.tensor_tensor(out_ap(hde), tv, out_ap(hde), ALU.add)


def _emit(tc, x_in, out_d, qk_dram, W):
    nc = tc.nc
    with (
        tc.tile_pool(name="consts", bufs=1) as consts,
        tc.tile_pool(name="xbf", bufs=1) as xbf_pool,
        tc.tile_pool(name="nat", bufs=2) as nat_pool,
        tc.tile_pool(name="tmp", bufs=1) as tmp_pool,
        tc.tile_pool(name="qkt", bufs=2) as qkt_pool,
        tc.tile_pool(name="vaug", bufs=3) as vaug_pool,
        tc.tile_pool(name="pt", bufs=6) as pt_pool,
        tc.tile_pool(name="small", bufs=4) as small_pool,
        tc.tile_pool(name="oacc", bufs=1) as oacc_pool,
        tc.tile_pool(name="stage", bufs=2, space="PSUM") as stage_pool,
        tc.tile_pool(name="ypsum", bufs=4, space="PSUM") as ypsum_pool,
    ):
        # ---- constants: strict-upper selector and MASK_NEG * I ----------
        ustrict = consts.tile([128, 128], BF16)
        nc.gpsimd.memset(ustrict, 1.0)
        nc.gpsimd.affine_select(
            out=ustrict, in_=ustrict, compare_op=ALU.is_gt, fill=0.0,
            base=0, pattern=[[1, 128]], channel_multiplier=-1,
        )
        negi = consts.tile([128, 128], BF16)
        nc.gpsimd.memset(negi, 0.0)
        nc.gpsimd.affine_select(
            out=negi, in_=negi, compare_op=ALU.not_equal, fill=MASK_NEG,
            base=0, pattern=[[-1, 128]], channel_multiplier=1,
        )
        scratch = consts.tile([128, 8], mybir.dt.float32)
        nc.gpsimd.memset(scratch[:, 0:1], 1.0)
        nc.scalar.activation(
            out=scratch[:, 1:2], in_=scratch[:, 0:1], func=ACTF.Exp,
            scale=1.0,
        )

        # ---- x loads happen in half-head column chunks, cast to bf16;
        # half-1 chunks are emitted mid-way through config 0's mixing so
        # config 0's bounce DMAs are not queued behind the whole 9.4MB ----
        xbf = xbf_pool.tile([128, NT, CIN], BF16)

        def load_x_half(half):
            for third in range(3):
                c0 = third * E + half * (E // 2)
                nc.gpsimd.dma_start(
                    out=xbf[:, :, c0 : c0 + E // 2],
                    in_=x_in[:, c0 : c0 + E // 2].rearrange(
                        "(a p) c -> p a c", p=128
                    ),
                )

        oacc = oacc_pool.tile([128, NT, E], F32)

        state = {}

        # weight order in W: for cfg ci, e in (384, 576, 768): W[3*ci + idx]
        def mix_config(ci):
            # generator: yields after each (half, tensor) piece so the
            # driver can interleave DVE mixing with the previous config's
            # attention normalizes (DVE executes in emission order)
            h = N_HEAD_LIST[ci]
            hd = E // h
            pw = _pw(h)
            h2 = h // 2
            scale = 1.0 / math.sqrt(hd)
            dchunks = _dchunks(h)
            ndt = h * pw // 128
            ndt2 = ndt // 2
            e_list = [(2, 768, hd), (1, 576, 576 // h), (0, 384, 384 // h)]

            # ---- mix + bounce + transpose per half of the heads ---------
            qkt = []
            vaug = vaug_pool.tile([128, NT, h, hd + 1], BF16, tag="vaug")
            for tensor_idx in range(2):
                tl = qkt_pool.tile(
                    [128, ndt, T], BF16, tag="qkt", bufs=4,
                    name=f"qkt{ci}{tensor_idx}",
                )
                qkt.append(tl)
            tmp = tmp_pool.tile([128, NT, 288], BF16, tag="tmp")
            tmpb = tmp_pool.tile([128, NT, 288], BF16, tag="tmpb")
            state[ci] = (qkt, vaug)
            for half in range(2):
                if ci == 0:
                    load_x_half(half)
                hsl = slice(half * h2, (half + 1) * h2)
                for tensor_idx in range(2):  # 0=Q (SP ring) 1=K (ACT ring)
                    base = tensor_idx * E
                    nat = nat_pool.tile(
                        [128, NT, h2, pw], BF16, tag="nat"
                    )
                    if pw > hd:
                        nc.vector.memset(nat[:, :, :, hd:pw], 0.0)

                    def xsrc(e, hde, base=base, half=half):
                        sl = xbf[
                            :, :,
                            base + half * (e // 2) : base + (half + 1) * (e // 2),
                        ]
                        return sl.rearrange("p a (h d) -> p a h d", h=h2)

                    def out_ap(hde, nat=nat):
                        return nat[:, :, :, 0:hde]

                    _mix_half(nc, W, ci, out_ap, xsrc, (tmp, tmpb), e_list, h2)

                    eng = nc.sync  # single HWDGE ring (dual-ring raced)
                    w0 = half * h2 * pw
                    wr = eng.dma_start(
                        out=qk_dram[ci][tensor_idx][
                            :, w0 : w0 + h2 * pw
                        ].rearrange("(a p) w -> p a w", p=128),
                        in_=nat[:, :, :, :],
                    )
                    for dt_ in range(half * ndt2, (half + 1) * ndt2):
                        rd = eng.dma_start(
                            out=qkt[tensor_idx][:, dt_, :],
                            in_=qk_dram[ci][tensor_idx][
                                :, dt_ * 128 : (dt_ + 1) * 128
                            ],
                            transpose=True,
                        )
                        add_dep_helper(
                            rd.ins, wr.ins, sync=True,
                            reason="dram bounce raw",
                        )
                    yield

                # V_aug for this half
                nc.vector.memset(vaug[:, :, hsl, hd : hd + 1], 1.0)

                def vsrc(e, hde, half=half):
                    sl = xbf[
                        :, :,
                        2 * E + half * (e // 2) : 2 * E + (half + 1) * (e // 2),
                    ]
                    return sl.rearrange("p a (h d) -> p a h d", h=h2)

                def vout(hde, hsl=hsl):
                    return vaug[:, :, hsl, 0:hde]

                _mix_half(nc, W, ci, vout, vsrc, (tmp, tmpb), e_list, h2,
                           add_eng=nc.gpsimd)
                yield

        def attn_config(ci):
            h = N_HEAD_LIST[ci]
            hd = E // h
            h2 = h // 2
            scale = 1.0 / math.sqrt(hd)
            dchunks = _dchunks(h)
            qkt, vaug = state.pop(ci)
            qt, kt = qkt

            # ---- attention ---------------------------------------------
            for s in range(2):
                ntk = 4 * s + 4
                for hf in range(2):
                    pheads = list(range(hf * h2, (hf + 1) * h2))
                    nh = h2
                    groups = [pheads[i : i + 2] for i in range(0, nh, 2)]
                    yts = [
                        ypsum_pool.tile(
                            [128, nh, hd + 1], F32, tag="y", name=f"yt{qt_}"
                        )
                        for qt_ in range(4)
                    ]
                    # One accumulation start per PSUM bank: start=True marks
                    # the whole 2KB zero region pending-zero, so only the
                    # first matmul in each Y bank carries it; later heads'
                    # first writes overwrite via the pending-zero bytes.
                    y_first = [None] * 4

                    def norm_qt(qt_):
                        tqg = 4 * s + qt_
                        lrow = small_pool.tile([128, 6], F32, tag="lrow")
                        rec = small_pool.tile([128, 6], F32, tag="rec")
                        nc.vector.tensor_copy(
                            lrow[:, 0:nh], yts[qt_][:, :, hd]
                        )
                        nc.vector.reciprocal(rec[:, 0:nh], lrow[:, 0:nh])
                        for jp, head in enumerate(pheads):
                            dst = oacc[:, tqg, head * hd : head * hd + hd]
                            if ci == 0:
                                nc.vector.tensor_scalar(
                                    dst, yts[qt_][:, jp, 0:hd],
                                    rec[:, jp : jp + 1], None, ALU.mult,
                                )
                            else:
                                nc.vector.scalar_tensor_tensor(
                                    out=dst,
                                    in0=yts[qt_][:, jp, 0:hd],
                                    scalar=rec[:, jp : jp + 1],
                                    in1=dst,
                                    op0=ALU.mult,
                                    op1=ALU.add,
                                )
                        if ci == 2 and hf == 1:
                            # this query tile is final: stream out
                            nc.sync.dma_start(
                                out=out_d[tqg * 128 : (tqg + 1) * 128, :],
                                in_=oacc[:, tqg, :],
                            )

                    def emit_pv(tk, g, ptl):
                        for qt_ in range(4):
                            qtg = 4 * s + qt_
                            if qtg < tk:
                                continue
                            for j, head in enumerate(g):
                                jp = head - hf * h2
                                is_start = tk == 0 and y_first[qt_] is None
                                mm = nc.tensor.matmul(
                                    out=yts[qt_][:, jp, :],
                                    lhsT=ptl[
                                        :, j, qt_ * 128 : (qt_ + 1) * 128
                                    ],
                                    rhs=vaug[:, tk, head, :],
                                    start=is_start,
                                    stop=(tk == qtg and jp == nh - 1),
                                )
                                if is_start:
                                    y_first[qt_] = mm
                                elif tk == 0:
                                    add_dep_helper(
                                        mm.ins,
                                        y_first[qt_].ins,
                                        reason="psum zero-region order",
                                    )
                        if g is groups[-1] and 0 <= tk - 4 * s < 4:
                            norm_qt(tk - 4 * s)

                    # software pipeline: emit S^T+exp for job i, then the
                    # PV matmuls of job i-1, so the in-order PE stream is
                    # never head-of-line blocked on the current exp
                    prev = None
                    for tk in range(ntk):
                        lo = max(0, tk * 128 - s * 512)
                        diag = tk >= 4 * s
                        dlo = tk * 128 - s * 512
                        for g in groups:
                            stage = stage_pool.tile(
                                [128, 2, 512], F32, tag="stage"
                            )
                            for j, head in enumerate(g):
                                chunks = dchunks[head]
                                n_mm = len(chunks) + (1 if diag else 0)
                                for mi, (a, b) in enumerate(chunks):
                                    nc.tensor.matmul(
                                        out=stage[:, j, lo:512],
                                        lhsT=kt[
                                            a % 128 : a % 128 + (b - a),
                                            a // 128,
                                            tk * 128 : (tk + 1) * 128,
                                        ],
                                        rhs=qt[
                                            a % 128 : a % 128 + (b - a),
                                            a // 128,
                                            s * 512 + lo : (s + 1) * 512,
                                        ],
                                        start=(mi == 0),
                                        stop=(mi == n_mm - 1),
                                    )
                                if diag:
                                    nc.tensor.matmul(
                                        out=stage[:, j, dlo : dlo + 128],
                                        lhsT=ustrict[:, :],
                                        rhs=negi[:, :],
                                        start=False,
                                        stop=True,
                                    )
                            ptl = pt_pool.tile([128, 2, 512], BF16, tag="pt")
                            nc.scalar.activation(
                                out=ptl[:, 0:2, lo:512],
                                in_=stage[:, 0:2, lo:512],
                                func=ACTF.Exp,
                                scale=scale,
                            )
                            if prev is not None:
                                emit_pv(*prev)
                            prev = (tk, g, ptl)
                    if prev is not None:
                        emit_pv(*prev)
                    yield

        def adv(gen, n=1):
            if gen is None:
                return None
            for _ in range(n):
                try:
                    next(gen)
                except StopIteration:
                    return None
            return gen

        for _ in mix_config(0):
            pass
        m_next = mix_config(1)
        for ci in range(3):
            a = attn_config(ci)
            while True:
                try:
                    next(a)
                except StopIteration:
                    break
                m_next = adv(m_next, 3)
            if m_next is not None:
                for _ in m_next:
                    pass
            m_next = mix_config(2) if ci == 0 else None


_PROGRAM_CACHE = {}


def _get_program(W):
    key = np.asarray(W, dtype=np.float32).tobytes()
    if key not in _PROGRAM_CACHE:
        _PROGRAM_CACHE[key] = _build_program(np.asarray(W, dtype=np.float32))
    return _PROGRAM_CACHE[key]


def kernel(x, weights):
    """x: [8, 1024, 2304] f32; weights: [9] f32 -> [8, 1024, 768] f32."""
    x = np.asarray(x, dtype=np.float32)
    weights = np.asarray(weights, dtype=np.float32)
    assert x.shape == (N_CORES, T, CIN), x.shape
    nc = _get_program(weights)
    in_maps = [{"x": np.ascontiguousarray(x[c])} for c in range(N_CORES)]
    res = run_bass_kernel_spmd(nc, in_maps, list(range(N_CORES)))
    return np.stack([res.results[c]["out"] for c in range(N_CORES)], axis=0)

